# revision 20
# baseline (speedup 1.0000x reference)
"""GAT (2-layer, 4-head) Trainium2 Bass kernel, 8-core SPMD.

Sharding: rows of the attention score matrix are sharded over cores
(core c owns output rows [c*BLK, (c+1)*BLK)).  Scores are computed in
transposed layout pT[j, i] (j on partitions) so the aggregation
matmul contracts j on the partition dim.  exp(leaky_relu(s)) is
computed exactly as max(exp(s), exp(0.2*s)); the softmax denominator
rides the aggregation matmul as a ones column.
"""

import os
import sys

import numpy as np

for _p in ("/opt/trn_rl_repo",):
    if _p not in sys.path and os.path.isdir(_p):
        sys.path.insert(0, _p)

import ml_dtypes

N = 4096
IN_DIM = 256
HID = 64
HEADS = 4
OUT_DIM = 64
NCORES = 8
BLK = N // NCORES  # rows per core

# Elementwise engine split knobs (fractions of the 32 j-chunks per layer).
ACT_FRAC = 0.75    # fraction of chunks on the ScalarE (Prelu+Exp) path
GP_FRAC = 0.0      # fraction of chunk-quads whose mask-multiply goes to GPSIMD
USE_PRELU = True   # Prelu+Exp on ACT path (HW only; CoreSim lacks Prelu)

BF16 = ml_dtypes.bfloat16


def _build_program(n=N, ncores=NCORES, act_frac=ACT_FRAC, gp_frac=GP_FRAC,
                   use_prelu=USE_PRELU):
    import concourse.bacc as bacc
    import concourse.bass as bass
    import concourse.mybir as mybir
    import concourse.tile as tile
    from concourse import masks

    blk = n // ncores
    njt = n // 128          # j partition-tiles
    nbt = blk // 128        # 128-row tiles inside the core's block
    f32 = mybir.dt.float32
    bf16 = mybir.dt.bfloat16
    Alu = mybir.AluOpType
    Act = mybir.ActivationFunctionType

    # quantize the ACT split to quads of 4 j-chunks
    n_act = 4 * int(round(njt * act_frac / 4))
    n_gp_q = int(round((njt // 4) * gp_frac))

    nc = bacc.Bacc("TRN2", target_bir_lowering=False, num_devices=ncores)

    # ---- I/O ----
    hT_d = nc.dram_tensor("hT", [IN_DIM, n], bf16, kind="ExternalInput")
    hTb_d = nc.dram_tensor("hTb", [IN_DIM, blk], bf16, kind="ExternalInput")
    wp_d = nc.dram_tensor("wp", [IN_DIM + 1, HEADS * HID + 2 * HEADS], bf16,
                          kind="ExternalInput")
    w2p_d = nc.dram_tensor("w2p", [HEADS * HID + 1, OUT_DIM + 2], bf16,
                           kind="ExternalInput")
    adjm_d = nc.dram_tensor("adjm", [n, blk], bf16, kind="ExternalInput")
    out_d = nc.dram_tensor("out_block", [blk, OUT_DIM], f32,
                           kind="ExternalOutput")

    WPC = HEADS * HID + 2 * HEADS  # 264

    with tile.TileContext(nc) as tc:
        with (
            tc.tile_pool(name="const", bufs=1) as cpool,
            tc.tile_pool(name="persist", bufs=1) as ppool,
            tc.tile_pool(name="work", bufs=2) as wpool,
            tc.tile_pool(name="dram", bufs=1, space="DRAM") as dpool,
        ):
            # ---- constant / persistent SBUF ----
            hT_sb = cpool.tile([128, 2, n], bf16)
            hTb_sb = cpool.tile([128, 2, blk], bf16)
            wp_sb = cpool.tile([128, 2, WPC], bf16)
            wpb_sb = cpool.tile([1, WPC], bf16)        # bias row of wp
            w2p_sb = cpool.tile([128, 2, OUT_DIM + 2], bf16)
            w2pb_sb = cpool.tile([1, OUT_DIM + 2], bf16)
            adj_sb = cpool.tile([128, njt, blk], bf16)
            ones_bf = cpool.tile([1, max(512, blk)], bf16)
            ones_f = cpool.tile([1, max(512, blk)], f32)
            ident = cpool.tile([128, 128], f32)

            nc.sync.dma_start(hT_sb[:], hT_d.rearrange("(c p) j -> p c j", p=128))
            nc.sync.dma_start(hTb_sb[:], hTb_d.rearrange("(c p) i -> p c i", p=128))
            nc.sync.dma_start(wp_sb[:], wp_d[0:IN_DIM, :].rearrange("(c p) m -> p c m", p=128))
            nc.sync.dma_start(wpb_sb[:], wp_d[IN_DIM:IN_DIM + 1, :])
            nc.sync.dma_start(w2p_sb[:], w2p_d[0:HEADS * HID, :].rearrange("(c p) m -> p c m", p=128))
            nc.sync.dma_start(w2pb_sb[:], w2p_d[HEADS * HID:HEADS * HID + 1, :])
            nc.sync.dma_start(adj_sb[:], adjm_d.rearrange("(jc p) i -> p jc i", p=128))
            nc.gpsimd.memset(ones_bf[:], 1.0)
            nc.gpsimd.memset(ones_f[:], 1.0)
            masks.make_identity(nc, ident[:])

            # wh for all heads, j-partitioned, with ones column per head
            # group layout [128, njt, HEADS, 66]: 64 wh | 1.0 | pad
            wh_sb = ppool.tile([128, njt, HEADS, 66], bf16)
            sd_sb = ppool.tile([128, njt, 2 * HEADS], f32)  # src/dst columns
            x2a_sb = ppool.tile([128, blk], bf16)   # x2T rows 0..127   (heads 0,1)
            x2b_sb = ppool.tile([128, blk], bf16)   # x2T rows 128..255 (heads 2,3)
            wh2_sb = ppool.tile([128, njt, 66], bf16)  # 64 wh2 | 1.0 | pad
            dst2_sb = ppool.tile([128, njt], f32)
            fin_sb = ppool.tile([64, blk], f32)
            ob_sb = ppool.tile([128, nbt, OUT_DIM], f32)
            agin_sb = ppool.tile([128, nbt, OUT_DIM + 2], f32)
            w2T_sb = ppool.tile([OUT_DIM + 2, blk], f32)

            nc.vector.memset(wh_sb[:, :, :, 64:66], 0.0)
            nc.vector.memset(wh_sb[:, :, :, 64:65], 1.0)
            nc.vector.memset(wh2_sb[:, :, 64:66], 0.0)
            nc.vector.memset(wh2_sb[:, :, 64:65], 1.0)

            ag_in = dpool.tile([blk, OUT_DIM + 2], f32)
            ag_out = nc.dram_tensor("ag_out", [n, OUT_DIM + 2], f32,
                                    addr_space="Shared")

            # ---- phase 1: wh + src/dst for all heads ----
            with tc.tile_pool(name="pwh", bufs=4, space="PSUM") as pwh:
                for jt in range(njt):
                    whp = pwh.tile([128, WPC], f32, tag="whp")
                    nc.tensor.matmul(whp[:], hT_sb[:, 0, jt * 128:(jt + 1) * 128],
                                     wp_sb[:, 0, :], start=True, stop=False)
                    nc.tensor.matmul(whp[:], hT_sb[:, 1, jt * 128:(jt + 1) * 128],
                                     wp_sb[:, 1, :], start=False, stop=False)
                    nc.tensor.matmul(whp[:], ones_bf[0:1, 0:128], wpb_sb[:],
                                     start=False, stop=True)
                    nc.vector.tensor_copy(
                        wh_sb[:, jt, :, 0:64],
                        whp[:, 0:HEADS * HID].rearrange("p (h d) -> p h d", h=HEADS))
                    nc.vector.tensor_copy(sd_sb[:, jt, :],
                                          whp[:, HEADS * HID:WPC])

            # ---- phase 2/3: attention layers ----
            with (
                tc.tile_pool(name="pout", bufs=2, space="PSUM") as pout,
                tc.tile_pool(name="pbc", bufs=2, space="PSUM") as pbc,
                tc.tile_pool(name="psm", bufs=1, space="PSUM") as psm,
            ):
                def score_loop(sbp, dst_all, dst_chunk, wh_chunk, out_dest):
                    """Masked-softmax attention for one head/layer.

                    sbp: [128, blk] f32 PSUM tile holding src_i broadcast
                    along partitions; dst_all: [128, njt] f32 AP of dst
                    values; dst_chunk(jc): [128,1] bias AP; wh_chunk(jc):
                    [128,65] lhsT AP; out_dest: [64, blk] AP for the
                    ELU'd normalized result.
                    """
                    A_b = wpool.tile([128, blk], bf16, tag="A_b")
                    a_b = wpool.tile([128, blk], bf16, tag="a_b")
                    B_t = wpool.tile([128, njt], f32, tag="B_t")
                    b_t = wpool.tile([128, njt], f32, tag="b_t")
                    if n_act < njt:
                        nc.scalar.activation(A_b[:], sbp[:], Act.Exp)
                        nc.scalar.activation(a_b[:], sbp[:], Act.Exp,
                                             scale=0.2)
                        nc.scalar.activation(B_t[:], dst_all, Act.Exp)
                        nc.scalar.activation(b_t[:], dst_all, Act.Exp,
                                             scale=0.2)
                    d02 = wpool.tile([128, njt], f32, tag="d02")
                    if n_act > 0 and not use_prelu:
                        nc.vector.tensor_scalar_mul(d02[:], dst_all, 0.2)

                    outp = pout.tile([65, blk], f32, tag="outp")
                    nquad = njt // 4
                    for q in range(nquad):
                        jcs = list(range(4 * q, 4 * q + 4))
                        t_q = wpool.tile([128, 4, blk], bf16, tag="t_q")
                        if jcs[0] < n_act:
                            # ScalarE path: exp(leaky(s)) per chunk
                            for k, jc in enumerate(jcs):
                                if use_prelu:
                                    l_t = wpool.tile([128, blk], f32,
                                                     tag="l_t")
                                    nc.scalar.activation(
                                        l_t[:], sbp[:], Act.Prelu,
                                        bias=dst_chunk(jc), alpha=0.2)
                                    nc.scalar.activation(
                                        t_q[:, k, :], l_t[:], Act.Exp)
                                else:
                                    e1 = wpool.tile([128, blk], bf16,
                                                    tag="e1")
                                    nc.scalar.activation(
                                        e1[:], sbp[:], Act.Exp,
                                        bias=dst_chunk(jc))
                                    nc.scalar.activation(
                                        t_q[:, k, :], sbp[:], Act.Exp,
                                        bias=d02[:, jc:jc + 1], scale=0.2)
                                    nc.vector.tensor_tensor(
                                        t_q[:, k, :], t_q[:, k, :], e1[:],
                                        Alu.max)
                        else:
                            # DVE path: rank-1 products + merged max
                            t1_q = wpool.tile([128, 4, blk], bf16,
                                              tag="t1_q")
                            for k, jc in enumerate(jcs):
                                nc.vector.tensor_scalar_mul(
                                    t1_q[:, k, :], A_b[:], B_t[:, jc:jc + 1])
                                nc.vector.tensor_scalar_mul(
                                    t_q[:, k, :], a_b[:], b_t[:, jc:jc + 1])
                            nc.vector.tensor_tensor(t_q[:], t_q[:], t1_q[:],
                                                    Alu.max)
                        # merged mask multiply over the whole quad
                        p_q = wpool.tile([128, 4, blk], bf16, tag="p_q")
                        adj_q = adj_sb[:, 4 * q:4 * q + 4, :]
                        if q < n_gp_q:
                            nc.gpsimd.tensor_tensor(p_q[:], t_q[:], adj_q,
                                                    Alu.mult)
                        else:
                            nc.vector.tensor_tensor(p_q[:], t_q[:], adj_q,
                                                    Alu.mult)
                        for k, jc in enumerate(jcs):
                            nc.tensor.matmul(outp[:], wh_chunk(jc),
                                             p_q[:, k, :],
                                             start=(jc == 0),
                                             stop=(jc == njt - 1))

                    # normalize + ELU
                    rinv = wpool.tile([1, blk], f32, tag="rinv")
                    nc.vector.reciprocal(rinv[:], outp[64:65, :])
                    rbp = psm.tile([64, blk], f32, tag="rb")
                    nc.tensor.matmul(rbp[:], ones_f[0:1, 0:64], rinv[:],
                                     start=True, stop=True)
                    rb_sb = wpool.tile([64, blk], f32, tag="rb_sb")
                    nc.vector.tensor_copy(rb_sb[:], rbp[:])
                    u_sb = wpool.tile([64, blk], f32, tag="u_sb")
                    nc.vector.tensor_tensor(u_sb[:], outp[0:64, :], rb_sb[:],
                                            Alu.mult)
                    e_u = wpool.tile([64, blk], f32, tag="e_u")
                    nc.scalar.activation(e_u[:], u_sb[:], Act.Exp)
                    m_t = wpool.tile([64, blk], f32, tag="m_t")
                    nc.vector.tensor_scalar(m_t[:], e_u[:], -1.0, 0.0,
                                            Alu.add, Alu.min)
                    nc.vector.scalar_tensor_tensor(out_dest, u_sb[:], 0.0,
                                                   m_t[:], Alu.max, Alu.add)

                def gat_scores(hname, src_col_w, src_col_b, dst_all,
                               dst_chunk, wh_chunk, out_dest):
                    # src row of this core's block, then broadcast via PE
                    srp = psm.tile([1, blk], f32, tag="srow")
                    nc.tensor.matmul(srp[:], src_col_w[0], hTb_sb[:, 0, :],
                                     start=True, stop=False)
                    nc.tensor.matmul(srp[:], src_col_w[1], hTb_sb[:, 1, :],
                                     start=False, stop=False)
                    nc.tensor.matmul(srp[:], src_col_b, ones_bf[0:1, 0:blk],
                                     start=False, stop=True)
                    sr_sb = wpool.tile([1, blk], f32, tag="srow_sb")
                    nc.vector.tensor_copy(sr_sb[:], srp[:])
                    sbp = pbc.tile([128, blk], f32, tag="sbc")
                    nc.tensor.matmul(sbp[:], ones_f[0:1, 0:128], sr_sb[:],
                                     start=True, stop=True)
                    score_loop(sbp, dst_all, dst_chunk, wh_chunk, out_dest)

                # ---- layer 1: 4 heads ----
                for h in range(HEADS):
                    cs = HEADS * HID + 2 * h  # src col in wp
                    x2_dest = (x2a_sb if h < 2 else x2b_sb)
                    gat_scores(
                        f"h{h}",
                        (wp_sb[:, 0, cs:cs + 1], wp_sb[:, 1, cs:cs + 1]),
                        wpb_sb[:, cs:cs + 1],
                        sd_sb[:, :, 2 * h + 1],
                        lambda jc, h=h: sd_sb[:, jc, 2 * h + 1:2 * h + 2],
                        lambda jc, h=h: wh_sb[:, jc, h, 0:65],
                        x2_dest[(h % 2) * 64:(h % 2) * 64 + 64, :],
                    )

                # ---- layer 2 projection + AllGather ----
                w2T = pout.tile([OUT_DIM + 2, blk], f32, tag="outp")
                nc.tensor.matmul(w2T[:], w2p_sb[:, 0, :], x2a_sb[:],
                                 start=True, stop=False)
                nc.tensor.matmul(w2T[:], w2p_sb[:, 1, :], x2b_sb[:],
                                 start=False, stop=False)
                nc.tensor.matmul(w2T[:], w2pb_sb[:], ones_bf[0:1, 0:blk],
                                 start=False, stop=True)
                nc.vector.tensor_copy(w2T_sb[:], w2T[:])
                for t in range(nbt):
                    trp = psm.tile([128, OUT_DIM + 2], f32, tag="trp")
                    nc.tensor.transpose(trp[:],
                                        w2T_sb[:, t * 128:(t + 1) * 128],
                                        ident[0:OUT_DIM + 2, 0:OUT_DIM + 2])
                    nc.vector.tensor_copy(agin_sb[:, t, :], trp[:])
                nc.sync.dma_start(ag_in.rearrange("(t p) m -> p t m", p=128),
                                  agin_sb[:])
                nc.gpsimd.collective_compute(
                    "AllGather", Alu.bypass,
                    replica_groups=[list(range(ncores))],
                    ins=[ag_in.opt()], outs=[ag_out[:]])
                agf_sb = cpool.tile([128, njt, OUT_DIM + 2], f32)
                nc.sync.dma_start(agf_sb[:],
                                  ag_out[:].rearrange("(jc p) m -> p jc m",
                                                      p=128))
                nc.vector.tensor_copy(wh2_sb[:, :, 0:64], agf_sb[:, :, 0:64])
                nc.vector.tensor_copy(dst2_sb[:, :],
                                      agf_sb[:, :, OUT_DIM + 1])

                # ---- layer 2 attention ----
                src2_sb = wpool.tile([1, blk], f32, tag="srow_sb")
                nc.vector.tensor_copy(src2_sb[:],
                                      w2T_sb[OUT_DIM:OUT_DIM + 1, :])

                srp2 = pbc.tile([128, blk], f32, tag="sbc")
                nc.tensor.matmul(srp2[:], ones_f[0:1, 0:128], src2_sb[:],
                                 start=True, stop=True)
                score_loop(srp2, dst2_sb[:, :],
                           lambda jc: dst2_sb[:, jc:jc + 1],
                           lambda jc: wh2_sb[:, jc, 0:65],
                           fin_sb[:])

                # ---- output: transpose back to [blk, 64] ----
                for t in range(nbt):
                    trp = psm.tile([128, OUT_DIM + 2], f32, tag="trp")
                    nc.tensor.transpose(trp[:, 0:OUT_DIM],
                                        fin_sb[:, t * 128:(t + 1) * 128],
                                        ident[0:64, 0:64])
                    nc.vector.tensor_copy(ob_sb[:, t, :], trp[:, 0:OUT_DIM])
                nc.sync.dma_start(out_d.rearrange("(t p) d -> p t d", p=128),
                                  ob_sb[:])

    nc.compile()
    return nc


def _host_prep(h, adj, Wh, bWh, a_src, a_dst, a_b, Wo, bWo, ao_src, ao_dst,
               ao_b, n=N, ncores=NCORES):
    """Build per-core input maps (numpy only)."""
    blk = n // ncores
    h = np.asarray(h, np.float32)
    adj = np.asarray(adj)

    hT = np.ascontiguousarray(h.T).astype(BF16)                    # [256, n]

    wp = np.zeros((IN_DIM + 1, HEADS * HID + 2 * HEADS), np.float32)
    for hd in range(HEADS):
        W = np.asarray(Wh[hd], np.float32)
        wp[0:IN_DIM, hd * HID:(hd + 1) * HID] = W
        wp[IN_DIM, hd * HID:(hd + 1) * HID] = np.asarray(bWh[hd], np.float32)
        asr = np.asarray(a_src[hd], np.float32)
        ads = np.asarray(a_dst[hd], np.float32)
        cs = HEADS * HID + 2 * hd
        wp[0:IN_DIM, cs] = W @ asr
        wp[IN_DIM, cs] = float(np.asarray(bWh[hd], np.float32) @ asr) + float(
            np.asarray(a_b[hd], np.float32))
        wp[0:IN_DIM, cs + 1] = W @ ads
        wp[IN_DIM, cs + 1] = float(np.asarray(bWh[hd], np.float32) @ ads)
    wp = wp.astype(BF16)

    Wo = np.asarray(Wo, np.float32)
    w2p = np.zeros((HEADS * HID + 1, OUT_DIM + 2), np.float32)
    w2p[0:HEADS * HID, 0:OUT_DIM] = Wo
    w2p[HEADS * HID, 0:OUT_DIM] = np.asarray(bWo, np.float32)
    aos = np.asarray(ao_src, np.float32)
    aod = np.asarray(ao_dst, np.float32)
    w2p[0:HEADS * HID, OUT_DIM] = Wo @ aos
    w2p[HEADS * HID, OUT_DIM] = float(np.asarray(bWo, np.float32) @ aos) + \
        float(np.asarray(ao_b, np.float32))
    w2p[0:HEADS * HID, OUT_DIM + 1] = Wo @ aod
    w2p[HEADS * HID, OUT_DIM + 1] = float(np.asarray(bWo, np.float32) @ aod)
    w2p = w2p.astype(BF16)

    adjf = (adj > 0).astype(np.float32)

    in_maps = []
    for c in range(ncores):
        sl = slice(c * blk, (c + 1) * blk)
        in_maps.append({
            "hT": hT,
            "hTb": np.ascontiguousarray(hT[:, sl]),
            "wp": wp,
            "w2p": w2p,
            "adjm": np.ascontiguousarray(adjf[sl, :].T).astype(BF16),
        })
    return in_maps


_NC_CACHE = {}


def _get_program(n=N, ncores=NCORES):
    key = (n, ncores, ACT_FRAC, GP_FRAC, USE_PRELU)
    if key not in _NC_CACHE:
        _NC_CACHE[key] = _build_program(n, ncores, ACT_FRAC, GP_FRAC,
                                        USE_PRELU)
    return _NC_CACHE[key]


def kernel(**inputs):
    from concourse.bass_utils import run_bass_kernel_spmd

    nc = _get_program()
    in_maps = _host_prep(**inputs)
    res = run_bass_kernel_spmd(nc, in_maps, core_ids=list(range(NCORES)))
    out = np.concatenate([r["out_block"] for r in res.results], axis=0)
    return out.astype(np.float32)


# ---------------------------------------------------------------------------
# Self-test on the CPU simulator with a small N (no hardware needed).
def _np_reference(h, adj, Wh, bWh, a_src, a_dst, a_b, Wo, bWo, ao_src,
                  ao_dst, ao_b):
    NEG = -9e15

    def layer(x, mask, W, b, asr, ads, ab):
        wh = x @ W + b
        s = (wh @ asr)[:, None] + (wh @ ads)[None, :] + ab
        s = np.where(s >= 0, s, 0.2 * s)
        s = np.where(mask, s, NEG)
        s = s - s.max(axis=1, keepdims=True)
        e = np.exp(s)
        att = e / e.sum(axis=1, keepdims=True)
        return att @ wh

    def elu(x):
        return np.where(x > 0, x, np.exp(np.minimum(x, 0)) - 1)

    mask = adj > 0
    heads = [layer(h, mask, Wh[i], bWh[i], a_src[i], a_dst[i], a_b[i])
             for i in range(HEADS)]
    x2 = np.concatenate([elu(hh) for hh in heads], axis=1)
    return elu(layer(x2, mask, Wo, bWo, ao_src, ao_dst, ao_b))


def _selftest(n=1024):
    from concourse.bass_interp import MultiCoreSim

    rng = np.random.default_rng(0)
    h = rng.standard_normal((n, IN_DIM)).astype(np.float32)
    adj = rng.integers(0, 2, (n, n)).astype(np.int32)
    Wh = (rng.standard_normal((HEADS, IN_DIM, HID)) * 0.08).astype(np.float32)
    bWh = (rng.standard_normal((HEADS, HID)) * 0.05).astype(np.float32)
    a_src = (rng.standard_normal((HEADS, HID)) * 0.1).astype(np.float32)
    a_dst = (rng.standard_normal((HEADS, HID)) * 0.1).astype(np.float32)
    a_b = (rng.standard_normal((HEADS,)) * 0.05).astype(np.float32)
    Wo = (rng.standard_normal((HEADS * HID, OUT_DIM)) * 0.06).astype(np.float32)
    bWo = (rng.standard_normal((OUT_DIM,)) * 0.05).astype(np.float32)
    ao_src = (rng.standard_normal((OUT_DIM,)) * 0.1).astype(np.float32)
    ao_dst = (rng.standard_normal((OUT_DIM,)) * 0.1).astype(np.float32)
    ao_b = np.float32(0.03)

    inputs = dict(h=h, adj=adj, Wh=Wh, bWh=bWh, a_src=a_src, a_dst=a_dst,
                  a_b=a_b, Wo=Wo, bWo=bWo, ao_src=ao_src, ao_dst=ao_dst,
                  ao_b=ao_b)
    expected = _np_reference(**inputs)

    nc = _build_program(n=n, ncores=NCORES, use_prelu=False)
    in_maps = _host_prep(n=n, ncores=NCORES, **inputs)
    sim = MultiCoreSim(nc, num_cores=NCORES)
    for c in range(NCORES):
        for k, v in in_maps[c].items():
            sim.cores[c].tensor(k)[:] = v
    sim.simulate(check_with_hw=False)
    out = np.concatenate(
        [np.asarray(sim.cores[c].tensor("out_block")) for c in range(NCORES)],
        axis=0)

    err = np.abs(out - expected)
    scale = np.abs(expected).max()
    print("max abs err:", err.max(), "scale:", scale,
          "rel:", err.max() / scale)
    return err.max() / scale


if __name__ == "__main__":
    _selftest(int(sys.argv[1]) if len(sys.argv) > 1 else 1024)


# revision 26
# speedup vs baseline: 1.0328x; 1.0328x over previous
"""GAT (2-layer, 4-head) Trainium2 Bass kernel, 8-core SPMD.

Sharding: rows of the attention score matrix are sharded over cores
(core c owns output rows [c*BLK, (c+1)*BLK)).  Scores are computed in
transposed layout pT[j, i] (j on partitions) so the aggregation
matmul contracts j on the partition dim.  exp(leaky_relu(s)) is
computed exactly as max(exp(s), exp(0.2*s)); the softmax denominator
rides the aggregation matmul as a ones column.
"""

import os
import sys

import numpy as np

for _p in ("/opt/trn_rl_repo",):
    if _p not in sys.path and os.path.isdir(_p):
        sys.path.insert(0, _p)

import ml_dtypes

N = 4096
IN_DIM = 256
HID = 64
HEADS = 4
OUT_DIM = 64
NCORES = 8
BLK = N // NCORES  # rows per core

# Elementwise engine split knobs (fractions of the 32 j-chunks per layer).
ACT_FRAC = 0.625   # fraction of chunks on the ScalarE (Prelu+Exp) path
GP_FRAC = 0.0      # fraction of chunk-quads whose mask-multiply goes to GPSIMD
USE_PRELU = True   # Prelu+Exp on ACT path (HW only; CoreSim lacks Prelu)

BF16 = ml_dtypes.bfloat16


def _build_program(n=N, ncores=NCORES, act_frac=ACT_FRAC, gp_frac=GP_FRAC,
                   use_prelu=USE_PRELU):
    import concourse.bacc as bacc
    import concourse.bass as bass
    import concourse.mybir as mybir
    import concourse.tile as tile
    from concourse import masks

    blk = n // ncores
    njt = n // 128          # j partition-tiles
    nbt = blk // 128        # 128-row tiles inside the core's block
    f32 = mybir.dt.float32
    bf16 = mybir.dt.bfloat16
    Alu = mybir.AluOpType
    Act = mybir.ActivationFunctionType

    # quantize the ACT split to quads of 4 j-chunks
    n_act = 4 * int(round(njt * act_frac / 4))
    n_gp_q = int(round((njt // 4) * gp_frac))

    nc = bacc.Bacc("TRN2", target_bir_lowering=False, num_devices=ncores)

    # ---- I/O ----
    hT_d = nc.dram_tensor("hT", [IN_DIM, n], bf16, kind="ExternalInput")
    hTb_d = nc.dram_tensor("hTb", [IN_DIM, blk], bf16, kind="ExternalInput")
    wp_d = nc.dram_tensor("wp", [IN_DIM + 1, HEADS * HID + 2 * HEADS], bf16,
                          kind="ExternalInput")
    w2p_d = nc.dram_tensor("w2p", [HEADS * HID + 1, OUT_DIM + 2], bf16,
                           kind="ExternalInput")
    adjm_d = nc.dram_tensor("adjm", [n, blk], bf16, kind="ExternalInput")
    out_d = nc.dram_tensor("out_block", [blk, OUT_DIM], f32,
                           kind="ExternalOutput")

    WPC = HEADS * HID + 2 * HEADS  # 264

    with tile.TileContext(nc) as tc:
        with (
            tc.tile_pool(name="const", bufs=1) as cpool,
            tc.tile_pool(name="persist", bufs=1) as ppool,
            tc.tile_pool(name="work", bufs=2) as wpool,
            tc.tile_pool(name="dram", bufs=1, space="DRAM") as dpool,
        ):
            # ---- constant / persistent SBUF ----
            hT_sb = cpool.tile([128, 2, n], bf16)
            hTb_sb = cpool.tile([128, 2, blk], bf16)
            wp_sb = cpool.tile([128, 2, WPC], bf16)
            wpb_sb = cpool.tile([1, WPC], bf16)        # bias row of wp
            w2p_sb = cpool.tile([128, 2, OUT_DIM + 2], bf16)
            w2pb_sb = cpool.tile([1, OUT_DIM + 2], bf16)
            adj_sb = cpool.tile([128, njt, blk], bf16)
            ones_bf = cpool.tile([1, max(512, blk)], bf16)
            ones_f = cpool.tile([1, max(512, blk)], f32)
            ident = cpool.tile([128, 128], f32)

            nc.sync.dma_start(wp_sb[:], wp_d[0:IN_DIM, :].rearrange("(c p) m -> p c m", p=128))
            nc.sync.dma_start(wpb_sb[:], wp_d[IN_DIM:IN_DIM + 1, :])
            nc.sync.dma_start(hTb_sb[:], hTb_d.rearrange("(c p) i -> p c i", p=128))
            hT_re = hT_d[:].rearrange("(c p) j -> p c j", p=128)
            for k in range(4):
                nc.sync.dma_start(hT_sb[:, :, k * (n // 4):(k + 1) * (n // 4)],
                                  hT_re[:, :, k * (n // 4):(k + 1) * (n // 4)])
            nc.sync.dma_start(w2p_sb[:], w2p_d[0:HEADS * HID, :].rearrange("(c p) m -> p c m", p=128))
            nc.sync.dma_start(w2pb_sb[:], w2p_d[HEADS * HID:HEADS * HID + 1, :])
            adj_re = adjm_d[:].rearrange("(jc p) i -> p jc i", p=128)
            for q in range(njt // 4):
                nc.sync.dma_start(adj_sb[:, 4 * q:4 * q + 4, :],
                                  adj_re[:, 4 * q:4 * q + 4, :])
            nc.gpsimd.memset(ones_bf[:], 1.0)
            nc.gpsimd.memset(ones_f[:], 1.0)
            masks.make_identity(nc, ident[:])

            # wh for all heads, j-partitioned, with ones column per head
            # group layout [128, njt, HEADS, 66]: 64 wh | 1.0 | pad
            wh_sb = ppool.tile([128, njt, HEADS, 66], bf16)
            sd_sb = ppool.tile([128, njt, 2 * HEADS], f32)  # src/dst columns
            x2a_sb = ppool.tile([128, blk], bf16)   # x2T rows 0..127   (heads 0,1)
            x2b_sb = ppool.tile([128, blk], bf16)   # x2T rows 128..255 (heads 2,3)
            wh2_sb = ppool.tile([128, njt, 66], bf16)  # 64 wh2 | 1.0 | pad
            dst2_sb = ppool.tile([128, njt], f32)
            fin_sb = ppool.tile([64, blk], f32)
            ob_sb = ppool.tile([128, nbt, OUT_DIM], f32)
            agin_sb = ppool.tile([128, nbt, OUT_DIM + 2], bf16)
            w2T_sb = ppool.tile([OUT_DIM + 2, blk], f32)

            nc.vector.memset(wh_sb[:, :, :, 64:66], 0.0)
            nc.vector.memset(wh_sb[:, :, :, 64:65], 1.0)
            nc.vector.memset(wh2_sb[:, :, 64:66], 0.0)
            nc.vector.memset(wh2_sb[:, :, 64:65], 1.0)

            ag_in = dpool.tile([blk, OUT_DIM + 2], bf16)
            ag_out = nc.dram_tensor("ag_out", [n, OUT_DIM + 2], bf16,
                                    addr_space="Shared")

            # ---- phase 1: wh + src/dst for all heads ----
            with tc.tile_pool(name="pwh", bufs=4, space="PSUM") as pwh:
                for jt in range(njt):
                    whp = pwh.tile([128, WPC], f32, tag="whp")
                    nc.tensor.matmul(whp[:], hT_sb[:, 0, jt * 128:(jt + 1) * 128],
                                     wp_sb[:, 0, :], start=True, stop=False)
                    nc.tensor.matmul(whp[:], hT_sb[:, 1, jt * 128:(jt + 1) * 128],
                                     wp_sb[:, 1, :], start=False, stop=False)
                    nc.tensor.matmul(whp[:], ones_bf[0:1, 0:128], wpb_sb[:],
                                     start=False, stop=True)
                    nc.vector.tensor_copy(
                        wh_sb[:, jt, :, 0:64],
                        whp[:, 0:HEADS * HID].rearrange("p (h d) -> p h d", h=HEADS))
                    nc.vector.tensor_copy(sd_sb[:, jt, :],
                                          whp[:, HEADS * HID:WPC])

            # ---- phase 2/3: attention layers ----
            with (
                tc.tile_pool(name="pout", bufs=2, space="PSUM") as pout,
                tc.tile_pool(name="pbc", bufs=2, space="PSUM") as pbc,
                tc.tile_pool(name="psm", bufs=1, space="PSUM") as psm,
            ):
                def score_loop(sbp, dst_all, dst_chunk, wh_chunk, out_dest):
                    """Masked-softmax attention for one head/layer.

                    sbp: [128, blk] f32 PSUM tile holding src_i broadcast
                    along partitions; dst_all: [128, njt] f32 AP of dst
                    values; dst_chunk(jc): [128,1] bias AP; wh_chunk(jc):
                    [128,65] lhsT AP; out_dest: [64, blk] AP for the
                    ELU'd normalized result.
                    """
                    A_b = wpool.tile([128, blk], bf16, tag="A_b")
                    a_b = wpool.tile([128, blk], bf16, tag="a_b")
                    B_t = wpool.tile([128, njt], f32, tag="B_t")
                    b_t = wpool.tile([128, njt], f32, tag="b_t")
                    if n_act < njt:
                        nc.scalar.activation(A_b[:], sbp[:], Act.Exp)
                        nc.scalar.activation(a_b[:], sbp[:], Act.Exp,
                                             scale=0.2)
                        nc.scalar.activation(B_t[:], dst_all, Act.Exp)
                        nc.scalar.activation(b_t[:], dst_all, Act.Exp,
                                             scale=0.2)
                    d02 = wpool.tile([128, njt], f32, tag="d02")
                    if n_act > 0 and not use_prelu:
                        nc.vector.tensor_scalar_mul(d02[:], dst_all, 0.2)

                    outp = pout.tile([65, blk], f32, tag="outp")
                    nquad = njt // 4
                    for q in range(nquad):
                        jcs = list(range(4 * q, 4 * q + 4))
                        t_q = wpool.tile([128, 4, blk], bf16, tag="t_q")
                        if jcs[0] < n_act:
                            # ScalarE path: exp(leaky(s)) per chunk
                            for k, jc in enumerate(jcs):
                                if use_prelu:
                                    l_t = wpool.tile([128, blk], f32,
                                                     tag="l_t")
                                    nc.scalar.activation(
                                        l_t[:], sbp[:], Act.Prelu,
                                        bias=dst_chunk(jc), alpha=0.2)
                                    nc.scalar.activation(
                                        t_q[:, k, :], l_t[:], Act.Exp)
                                else:
                                    e1 = wpool.tile([128, blk], bf16,
                                                    tag="e1")
                                    nc.scalar.activation(
                                        e1[:], sbp[:], Act.Exp,
                                        bias=dst_chunk(jc))
                                    nc.scalar.activation(
                                        t_q[:, k, :], sbp[:], Act.Exp,
                                        bias=d02[:, jc:jc + 1], scale=0.2)
                                    nc.vector.tensor_tensor(
                                        t_q[:, k, :], t_q[:, k, :], e1[:],
                                        Alu.max)
                        else:
                            # DVE path: rank-1 products + merged max
                            t1_q = wpool.tile([128, 4, blk], bf16,
                                              tag="t1_q")
                            for k, jc in enumerate(jcs):
                                nc.vector.tensor_scalar_mul(
                                    t1_q[:, k, :], A_b[:], B_t[:, jc:jc + 1])
                                nc.vector.tensor_scalar_mul(
                                    t_q[:, k, :], a_b[:], b_t[:, jc:jc + 1])
                            nc.vector.tensor_tensor(t_q[:], t_q[:], t1_q[:],
                                                    Alu.max)
                        # merged mask multiply over the whole quad
                        p_q = wpool.tile([128, 4, blk], bf16, tag="p_q")
                        adj_q = adj_sb[:, 4 * q:4 * q + 4, :]
                        if q < n_gp_q:
                            nc.gpsimd.tensor_tensor(p_q[:], t_q[:], adj_q,
                                                    Alu.mult)
                        else:
                            nc.vector.tensor_tensor(p_q[:], t_q[:], adj_q,
                                                    Alu.mult)
                        for k, jc in enumerate(jcs):
                            nc.tensor.matmul(outp[:], wh_chunk(jc),
                                             p_q[:, k, :],
                                             start=(jc == 0),
                                             stop=(jc == njt - 1))

                    # normalize + ELU
                    rinv = wpool.tile([1, blk], f32, tag="rinv")
                    nc.vector.reciprocal(rinv[:], outp[64:65, :])
                    rbp = psm.tile([64, blk], f32, tag="rb")
                    nc.tensor.matmul(rbp[:], ones_f[0:1, 0:64], rinv[:],
                                     start=True, stop=True)
                    rb_sb = wpool.tile([64, blk], f32, tag="rb_sb")
                    nc.vector.tensor_copy(rb_sb[:], rbp[:])
                    u_sb = wpool.tile([64, blk], f32, tag="u_sb")
                    nc.vector.tensor_tensor(u_sb[:], outp[0:64, :], rb_sb[:],
                                            Alu.mult)
                    e_u = wpool.tile([64, blk], f32, tag="e_u")
                    nc.scalar.activation(e_u[:], u_sb[:], Act.Exp)
                    m_t = wpool.tile([64, blk], f32, tag="m_t")
                    nc.vector.tensor_scalar(m_t[:], e_u[:], -1.0, 0.0,
                                            Alu.add, Alu.min)
                    nc.vector.scalar_tensor_tensor(out_dest, u_sb[:], 0.0,
                                                   m_t[:], Alu.max, Alu.add)

                def gat_scores(hname, src_col_w, src_col_b, dst_all,
                               dst_chunk, wh_chunk, out_dest):
                    # src row of this core's block, then broadcast via PE
                    srp = psm.tile([1, blk], f32, tag="srow")
                    nc.tensor.matmul(srp[:], src_col_w[0], hTb_sb[:, 0, :],
                                     start=True, stop=False)
                    nc.tensor.matmul(srp[:], src_col_w[1], hTb_sb[:, 1, :],
                                     start=False, stop=False)
                    nc.tensor.matmul(srp[:], src_col_b, ones_bf[0:1, 0:blk],
                                     start=False, stop=True)
                    sr_sb = wpool.tile([1, blk], f32, tag="srow_sb")
                    nc.vector.tensor_copy(sr_sb[:], srp[:])
                    sbp = pbc.tile([128, blk], f32, tag="sbc")
                    nc.tensor.matmul(sbp[:], ones_f[0:1, 0:128], sr_sb[:],
                                     start=True, stop=True)
                    score_loop(sbp, dst_all, dst_chunk, wh_chunk, out_dest)

                # ---- layer 1: 4 heads ----
                for h in range(HEADS):
                    cs = HEADS * HID + 2 * h  # src col in wp
                    x2_dest = (x2a_sb if h < 2 else x2b_sb)
                    gat_scores(
                        f"h{h}",
                        (wp_sb[:, 0, cs:cs + 1], wp_sb[:, 1, cs:cs + 1]),
                        wpb_sb[:, cs:cs + 1],
                        sd_sb[:, :, 2 * h + 1],
                        lambda jc, h=h: sd_sb[:, jc, 2 * h + 1:2 * h + 2],
                        lambda jc, h=h: wh_sb[:, jc, h, 0:65],
                        x2_dest[(h % 2) * 64:(h % 2) * 64 + 64, :],
                    )

                # ---- layer 2 projection + AllGather ----
                w2T = pout.tile([OUT_DIM + 2, blk], f32, tag="outp")
                nc.tensor.matmul(w2T[:], w2p_sb[:, 0, :], x2a_sb[:],
                                 start=True, stop=False)
                nc.tensor.matmul(w2T[:], w2p_sb[:, 1, :], x2b_sb[:],
                                 start=False, stop=False)
                nc.tensor.matmul(w2T[:], w2pb_sb[:], ones_bf[0:1, 0:blk],
                                 start=False, stop=True)
                nc.vector.tensor_copy(w2T_sb[:], w2T[:])
                for t in range(nbt):
                    trp = psm.tile([128, OUT_DIM + 2], f32, tag="trp")
                    nc.tensor.transpose(trp[:],
                                        w2T_sb[:, t * 128:(t + 1) * 128],
                                        ident[0:OUT_DIM + 2, 0:OUT_DIM + 2])
                    nc.vector.tensor_copy(agin_sb[:, t, :], trp[:])
                nc.sync.dma_start(ag_in.rearrange("(t p) m -> p t m", p=128),
                                  agin_sb[:])
                nc.gpsimd.collective_compute(
                    "AllGather", Alu.bypass,
                    replica_groups=[list(range(ncores))],
                    ins=[ag_in.opt()], outs=[ag_out[:]])
                agf_sb = cpool.tile([128, njt, OUT_DIM + 2], bf16)
                nc.sync.dma_start(agf_sb[:],
                                  ag_out[:].rearrange("(jc p) m -> p jc m",
                                                      p=128))
                nc.vector.tensor_copy(wh2_sb[:, :, 0:64], agf_sb[:, :, 0:64])
                nc.vector.tensor_copy(dst2_sb[:, :],
                                      agf_sb[:, :, OUT_DIM + 1])

                # ---- layer 2 attention ----
                src2_sb = wpool.tile([1, blk], f32, tag="srow_sb")
                nc.vector.tensor_copy(src2_sb[:],
                                      w2T_sb[OUT_DIM:OUT_DIM + 1, :])

                srp2 = pbc.tile([128, blk], f32, tag="sbc")
                nc.tensor.matmul(srp2[:], ones_f[0:1, 0:128], src2_sb[:],
                                 start=True, stop=True)
                score_loop(srp2, dst2_sb[:, :],
                           lambda jc: dst2_sb[:, jc:jc + 1],
                           lambda jc: wh2_sb[:, jc, 0:65],
                           fin_sb[:])

                # ---- output: transpose back to [blk, 64] ----
                for t in range(nbt):
                    trp = psm.tile([128, OUT_DIM + 2], f32, tag="trp")
                    nc.tensor.transpose(trp[:, 0:OUT_DIM],
                                        fin_sb[:, t * 128:(t + 1) * 128],
                                        ident[0:64, 0:64])
                    nc.vector.tensor_copy(ob_sb[:, t, :], trp[:, 0:OUT_DIM])
                nc.sync.dma_start(out_d.rearrange("(t p) d -> p t d", p=128),
                                  ob_sb[:])

    nc.compile()
    return nc


def _host_prep(h, adj, Wh, bWh, a_src, a_dst, a_b, Wo, bWo, ao_src, ao_dst,
               ao_b, n=N, ncores=NCORES):
    """Build per-core input maps (numpy only)."""
    blk = n // ncores
    h = np.asarray(h, np.float32)
    adj = np.asarray(adj)

    hT = np.ascontiguousarray(h.T).astype(BF16)                    # [256, n]

    wp = np.zeros((IN_DIM + 1, HEADS * HID + 2 * HEADS), np.float32)
    for hd in range(HEADS):
        W = np.asarray(Wh[hd], np.float32)
        wp[0:IN_DIM, hd * HID:(hd + 1) * HID] = W
        wp[IN_DIM, hd * HID:(hd + 1) * HID] = np.asarray(bWh[hd], np.float32)
        asr = np.asarray(a_src[hd], np.float32)
        ads = np.asarray(a_dst[hd], np.float32)
        cs = HEADS * HID + 2 * hd
        wp[0:IN_DIM, cs] = W @ asr
        wp[IN_DIM, cs] = float(np.asarray(bWh[hd], np.float32) @ asr) + float(
            np.asarray(a_b[hd], np.float32))
        wp[0:IN_DIM, cs + 1] = W @ ads
        wp[IN_DIM, cs + 1] = float(np.asarray(bWh[hd], np.float32) @ ads)
    wp = wp.astype(BF16)

    Wo = np.asarray(Wo, np.float32)
    w2p = np.zeros((HEADS * HID + 1, OUT_DIM + 2), np.float32)
    w2p[0:HEADS * HID, 0:OUT_DIM] = Wo
    w2p[HEADS * HID, 0:OUT_DIM] = np.asarray(bWo, np.float32)
    aos = np.asarray(ao_src, np.float32)
    aod = np.asarray(ao_dst, np.float32)
    w2p[0:HEADS * HID, OUT_DIM] = Wo @ aos
    w2p[HEADS * HID, OUT_DIM] = float(np.asarray(bWo, np.float32) @ aos) + \
        float(np.asarray(ao_b, np.float32))
    w2p[0:HEADS * HID, OUT_DIM + 1] = Wo @ aod
    w2p[HEADS * HID, OUT_DIM + 1] = float(np.asarray(bWo, np.float32) @ aod)
    w2p = w2p.astype(BF16)

    adjf = (adj > 0).astype(np.float32)

    in_maps = []
    for c in range(ncores):
        sl = slice(c * blk, (c + 1) * blk)
        in_maps.append({
            "hT": hT,
            "hTb": np.ascontiguousarray(hT[:, sl]),
            "wp": wp,
            "w2p": w2p,
            "adjm": np.ascontiguousarray(adjf[sl, :].T).astype(BF16),
        })
    return in_maps


_NC_CACHE = {}


def _get_program(n=N, ncores=NCORES):
    key = (n, ncores, ACT_FRAC, GP_FRAC, USE_PRELU)
    if key not in _NC_CACHE:
        _NC_CACHE[key] = _build_program(n, ncores, ACT_FRAC, GP_FRAC,
                                        USE_PRELU)
    return _NC_CACHE[key]


def kernel(**inputs):
    from concourse.bass_utils import run_bass_kernel_spmd

    nc = _get_program()
    in_maps = _host_prep(**inputs)
    res = run_bass_kernel_spmd(nc, in_maps, core_ids=list(range(NCORES)))
    out = np.concatenate([r["out_block"] for r in res.results], axis=0)
    return out.astype(np.float32)


# ---------------------------------------------------------------------------
# Self-test on the CPU simulator with a small N (no hardware needed).
def _np_reference(h, adj, Wh, bWh, a_src, a_dst, a_b, Wo, bWo, ao_src,
                  ao_dst, ao_b):
    NEG = -9e15

    def layer(x, mask, W, b, asr, ads, ab):
        wh = x @ W + b
        s = (wh @ asr)[:, None] + (wh @ ads)[None, :] + ab
        s = np.where(s >= 0, s, 0.2 * s)
        s = np.where(mask, s, NEG)
        s = s - s.max(axis=1, keepdims=True)
        e = np.exp(s)
        att = e / e.sum(axis=1, keepdims=True)
        return att @ wh

    def elu(x):
        return np.where(x > 0, x, np.exp(np.minimum(x, 0)) - 1)

    mask = adj > 0
    heads = [layer(h, mask, Wh[i], bWh[i], a_src[i], a_dst[i], a_b[i])
             for i in range(HEADS)]
    x2 = np.concatenate([elu(hh) for hh in heads], axis=1)
    return elu(layer(x2, mask, Wo, bWo, ao_src, ao_dst, ao_b))


def _selftest(n=1024):
    from concourse.bass_interp import MultiCoreSim

    rng = np.random.default_rng(0)
    h = rng.standard_normal((n, IN_DIM)).astype(np.float32)
    adj = rng.integers(0, 2, (n, n)).astype(np.int32)
    Wh = (rng.standard_normal((HEADS, IN_DIM, HID)) * 0.08).astype(np.float32)
    bWh = (rng.standard_normal((HEADS, HID)) * 0.05).astype(np.float32)
    a_src = (rng.standard_normal((HEADS, HID)) * 0.1).astype(np.float32)
    a_dst = (rng.standard_normal((HEADS, HID)) * 0.1).astype(np.float32)
    a_b = (rng.standard_normal((HEADS,)) * 0.05).astype(np.float32)
    Wo = (rng.standard_normal((HEADS * HID, OUT_DIM)) * 0.06).astype(np.float32)
    bWo = (rng.standard_normal((OUT_DIM,)) * 0.05).astype(np.float32)
    ao_src = (rng.standard_normal((OUT_DIM,)) * 0.1).astype(np.float32)
    ao_dst = (rng.standard_normal((OUT_DIM,)) * 0.1).astype(np.float32)
    ao_b = np.float32(0.03)

    inputs = dict(h=h, adj=adj, Wh=Wh, bWh=bWh, a_src=a_src, a_dst=a_dst,
                  a_b=a_b, Wo=Wo, bWo=bWo, ao_src=ao_src, ao_dst=ao_dst,
                  ao_b=ao_b)
    expected = _np_reference(**inputs)

    nc = _build_program(n=n, ncores=NCORES, use_prelu=False)
    in_maps = _host_prep(n=n, ncores=NCORES, **inputs)
    sim = MultiCoreSim(nc, num_cores=NCORES)
    for c in range(NCORES):
        for k, v in in_maps[c].items():
            sim.cores[c].tensor(k)[:] = v
    sim.simulate(check_with_hw=False)
    out = np.concatenate(
        [np.asarray(sim.cores[c].tensor("out_block")) for c in range(NCORES)],
        axis=0)

    err = np.abs(out - expected)
    scale = np.abs(expected).max()
    print("max abs err:", err.max(), "scale:", scale,
          "rel:", err.max() / scale)
    return err.max() / scale


if __name__ == "__main__":
    _selftest(int(sys.argv[1]) if len(sys.argv) > 1 else 1024)


# revision 29
# speedup vs baseline: 1.0633x; 1.0295x over previous
"""GAT (2-layer, 4-head) Trainium2 Bass kernel, 8-core SPMD.

Sharding: rows of the attention score matrix are sharded over cores
(core c owns output rows [c*BLK, (c+1)*BLK)).  Scores are computed in
transposed layout pT[j, i] (j on partitions) so the aggregation
matmul contracts j on the partition dim.  exp(leaky_relu(s)) is
computed exactly as max(exp(s), exp(0.2*s)); the softmax denominator
rides the aggregation matmul as a ones column.
"""

import os
import sys

import numpy as np

for _p in ("/opt/trn_rl_repo",):
    if _p not in sys.path and os.path.isdir(_p):
        sys.path.insert(0, _p)

import ml_dtypes

N = 4096
IN_DIM = 256
HID = 64
HEADS = 4
OUT_DIM = 64
NCORES = 8
BLK = N // NCORES  # rows per core

# Elementwise engine split knobs (fractions of the 32 j-chunks per layer).
ACT_FRAC = 0.69    # fraction of chunks on the ScalarE (Prelu+Exp) path
GP_FRAC = 0.0      # fraction of chunk-quads whose mask-multiply goes to GPSIMD
USE_PRELU = True   # Prelu+Exp on ACT path (HW only; CoreSim lacks Prelu)

BF16 = ml_dtypes.bfloat16


def _build_program(n=N, ncores=NCORES, act_frac=ACT_FRAC, gp_frac=GP_FRAC,
                   use_prelu=USE_PRELU):
    import concourse.bacc as bacc
    import concourse.bass as bass
    import concourse.mybir as mybir
    import concourse.tile as tile
    from concourse import masks

    blk = n // ncores
    njt = n // 128          # j partition-tiles
    nbt = blk // 128        # 128-row tiles inside the core's block
    f32 = mybir.dt.float32
    bf16 = mybir.dt.bfloat16
    Alu = mybir.AluOpType
    Act = mybir.ActivationFunctionType

    n_act = int(round(njt * act_frac))
    n_gp_q = int(round((njt // 4) * gp_frac))

    nc = bacc.Bacc("TRN2", target_bir_lowering=False, num_devices=ncores)

    # ---- I/O ----
    hT_d = nc.dram_tensor("hT", [IN_DIM, n], bf16, kind="ExternalInput")
    hTb_d = nc.dram_tensor("hTb", [IN_DIM, blk], bf16, kind="ExternalInput")
    wp_d = nc.dram_tensor("wp", [IN_DIM + 1, HEADS * HID + 2 * HEADS], bf16,
                          kind="ExternalInput")
    w2p_d = nc.dram_tensor("w2p", [HEADS * HID + 1, OUT_DIM + 2], bf16,
                           kind="ExternalInput")
    adjm_d = nc.dram_tensor("adjm", [n, blk], bf16, kind="ExternalInput")
    out_d = nc.dram_tensor("out_block", [blk, OUT_DIM], f32,
                           kind="ExternalOutput")

    WPC = HEADS * HID + 2 * HEADS  # 264

    with tile.TileContext(nc) as tc:
        with (
            tc.tile_pool(name="const", bufs=1) as cpool,
            tc.tile_pool(name="persist", bufs=1) as ppool,
            tc.tile_pool(name="work", bufs=2) as wpool,
            tc.tile_pool(name="dram", bufs=1, space="DRAM") as dpool,
        ):
            # ---- constant / persistent SBUF ----
            hT_sb = cpool.tile([128, 2, n], bf16)
            hTb_sb = cpool.tile([128, 2, blk], bf16)
            wp_sb = cpool.tile([128, 2, WPC], bf16)
            wpb_sb = cpool.tile([1, WPC], bf16)        # bias row of wp
            w2p_sb = cpool.tile([128, 2, OUT_DIM + 2], bf16)
            w2pb_sb = cpool.tile([1, OUT_DIM + 2], bf16)
            adj_sb = cpool.tile([128, njt, blk], bf16)
            ones_bf = cpool.tile([1, max(512, blk)], bf16)
            ones_f = cpool.tile([1, max(512, blk)], f32)
            ident = cpool.tile([128, 128], f32)

            nc.sync.dma_start(wp_sb[:], wp_d[0:IN_DIM, :].rearrange("(c p) m -> p c m", p=128))
            nc.sync.dma_start(wpb_sb[:], wp_d[IN_DIM:IN_DIM + 1, :])
            nc.sync.dma_start(hTb_sb[:], hTb_d.rearrange("(c p) i -> p c i", p=128))
            hT_re = hT_d[:].rearrange("(c p) j -> p c j", p=128)
            for k in range(4):
                nc.sync.dma_start(hT_sb[:, :, k * (n // 4):(k + 1) * (n // 4)],
                                  hT_re[:, :, k * (n // 4):(k + 1) * (n // 4)])
            nc.sync.dma_start(w2p_sb[:], w2p_d[0:HEADS * HID, :].rearrange("(c p) m -> p c m", p=128))
            nc.sync.dma_start(w2pb_sb[:], w2p_d[HEADS * HID:HEADS * HID + 1, :])
            adj_re = adjm_d[:].rearrange("(jc p) i -> p jc i", p=128)
            for q in range(njt // 4):
                nc.sync.dma_start(adj_sb[:, 4 * q:4 * q + 4, :],
                                  adj_re[:, 4 * q:4 * q + 4, :])
            nc.gpsimd.memset(ones_bf[:], 1.0)
            nc.gpsimd.memset(ones_f[:], 1.0)
            masks.make_identity(nc, ident[:])

            # wh for all heads, j-partitioned, with ones column per head
            # group layout [128, njt, HEADS, 66]: 64 wh | 1.0 | pad
            wh_sb = ppool.tile([128, njt, HEADS, 66], bf16)
            sd_sb = ppool.tile([128, njt, 2 * HEADS], f32)  # src/dst columns
            x2a_sb = ppool.tile([128, blk], bf16)   # x2T rows 0..127   (heads 0,1)
            x2b_sb = ppool.tile([128, blk], bf16)   # x2T rows 128..255 (heads 2,3)
            wh2_sb = ppool.tile([128, njt, 66], bf16)  # 64 wh2 | 1.0 | pad
            dst2_sb = ppool.tile([128, njt], f32)
            fin_sb = ppool.tile([64, blk], f32)
            ob_sb = ppool.tile([128, nbt, OUT_DIM], f32)
            agin_sb = ppool.tile([128, nbt, OUT_DIM + 2], bf16)
            w2T_sb = ppool.tile([OUT_DIM + 2, blk], f32)

            nc.vector.memset(wh_sb[:, :, :, 64:66], 0.0)
            nc.vector.memset(wh_sb[:, :, :, 64:65], 1.0)
            nc.vector.memset(wh2_sb[:, :, 64:66], 0.0)
            nc.vector.memset(wh2_sb[:, :, 64:65], 1.0)

            ag_in = dpool.tile([blk, OUT_DIM + 2], bf16)
            ag_out = nc.dram_tensor("ag_out", [n, OUT_DIM + 2], bf16,
                                    addr_space="Shared")

            # ---- phase 1: wh + src/dst for all heads ----
            with tc.tile_pool(name="pwh", bufs=4, space="PSUM") as pwh:
                for jt in range(njt):
                    whp = pwh.tile([128, WPC], f32, tag="whp")
                    nc.tensor.matmul(whp[:], hT_sb[:, 0, jt * 128:(jt + 1) * 128],
                                     wp_sb[:, 0, :], start=True, stop=False)
                    nc.tensor.matmul(whp[:], hT_sb[:, 1, jt * 128:(jt + 1) * 128],
                                     wp_sb[:, 1, :], start=False, stop=False)
                    nc.tensor.matmul(whp[:], ones_bf[0:1, 0:128], wpb_sb[:],
                                     start=False, stop=True)
                    nc.vector.tensor_copy(
                        wh_sb[:, jt, :, 0:64],
                        whp[:, 0:HEADS * HID].rearrange("p (h d) -> p h d", h=HEADS))
                    nc.vector.tensor_copy(sd_sb[:, jt, :],
                                          whp[:, HEADS * HID:WPC])

            # ---- phase 2/3: attention layers ----
            with (
                tc.tile_pool(name="pout", bufs=2, space="PSUM") as pout,
                tc.tile_pool(name="pbc", bufs=2, space="PSUM") as pbc,
                tc.tile_pool(name="psm", bufs=1, space="PSUM") as psm,
            ):
                def score_loop(sbp, dst_all, dst_chunk, wh_chunk, out_dest):
                    """Masked-softmax attention for one head/layer.

                    sbp: [128, blk] f32 PSUM tile holding src_i broadcast
                    along partitions; dst_all: [128, njt] f32 AP of dst
                    values; dst_chunk(jc): [128,1] bias AP; wh_chunk(jc):
                    [128,65] lhsT AP; out_dest: [64, blk] AP for the
                    ELU'd normalized result.
                    """
                    A_b = wpool.tile([128, blk], bf16, tag="A_b")
                    a_b = wpool.tile([128, blk], bf16, tag="a_b")
                    B_t = wpool.tile([128, njt], f32, tag="B_t")
                    b_t = wpool.tile([128, njt], f32, tag="b_t")
                    d02 = wpool.tile([128, njt], f32, tag="d02")
                    if n_act > 0 and not use_prelu:
                        nc.vector.tensor_scalar_mul(d02[:], dst_all, 0.2)
                    dve_setup = [False]

                    outp = pout.tile([65, blk], f32, tag="outp")
                    nquad = njt // 4
                    for q in range(nquad):
                        jcs = list(range(4 * q, 4 * q + 4))
                        t_q = wpool.tile([128, 4, blk], bf16, tag="t_q")
                        n_dve_k = sum(1 for jc in jcs if jc >= n_act)
                        if n_dve_k and not dve_setup[0]:
                            # emitted lazily: B_t/b_t depend on the whole
                            # wh phase; the Prelu chunks don't
                            nc.scalar.activation(A_b[:], sbp[:], Act.Exp)
                            nc.scalar.activation(a_b[:], sbp[:], Act.Exp,
                                                 scale=0.2)
                            nc.scalar.activation(B_t[:], dst_all, Act.Exp)
                            nc.scalar.activation(b_t[:], dst_all, Act.Exp,
                                                 scale=0.2)
                            dve_setup[0] = True
                        t1_q = None
                        for k, jc in enumerate(jcs):
                            if jc < n_act:
                                # ScalarE path: exp(leaky(s))
                                if use_prelu:
                                    l_t = wpool.tile([128, blk], f32,
                                                     tag="l_t")
                                    nc.scalar.activation(
                                        l_t[:], sbp[:], Act.Prelu,
                                        bias=dst_chunk(jc), alpha=0.2)
                                    nc.scalar.activation(
                                        t_q[:, k, :], l_t[:], Act.Exp)
                                else:
                                    e1 = wpool.tile([128, blk], bf16,
                                                    tag="e1")
                                    nc.scalar.activation(
                                        e1[:], sbp[:], Act.Exp,
                                        bias=dst_chunk(jc))
                                    nc.scalar.activation(
                                        t_q[:, k, :], sbp[:], Act.Exp,
                                        bias=d02[:, jc:jc + 1], scale=0.2)
                                    nc.vector.tensor_tensor(
                                        t_q[:, k, :], t_q[:, k, :], e1[:],
                                        Alu.max)
                            else:
                                # DVE path: rank-1 products, max merged below
                                if t1_q is None:
                                    t1_q = wpool.tile([128, 4, blk], bf16,
                                                      tag="t1_q")
                                nc.vector.tensor_scalar_mul(
                                    t1_q[:, k, :], A_b[:], B_t[:, jc:jc + 1])
                                nc.vector.tensor_scalar_mul(
                                    t_q[:, k, :], a_b[:], b_t[:, jc:jc + 1])
                        if n_dve_k:
                            k0 = 4 - n_dve_k
                            nc.vector.tensor_tensor(
                                t_q[:, k0:4, :], t_q[:, k0:4, :],
                                t1_q[:, k0:4, :], Alu.max)
                        # merged mask multiply over the whole quad
                        p_q = wpool.tile([128, 4, blk], bf16, tag="p_q")
                        adj_q = adj_sb[:, 4 * q:4 * q + 4, :]
                        if q < n_gp_q:
                            nc.gpsimd.tensor_tensor(p_q[:], t_q[:], adj_q,
                                                    Alu.mult)
                        else:
                            nc.vector.tensor_tensor(p_q[:], t_q[:], adj_q,
                                                    Alu.mult)
                        for k, jc in enumerate(jcs):
                            nc.tensor.matmul(outp[:], wh_chunk(jc),
                                             p_q[:, k, :],
                                             start=(jc == 0),
                                             stop=(jc == njt - 1))

                    # normalize + ELU
                    rinv = wpool.tile([1, blk], f32, tag="rinv")
                    nc.vector.reciprocal(rinv[:], outp[64:65, :])
                    rbp = psm.tile([64, blk], f32, tag="rb")
                    nc.tensor.matmul(rbp[:], ones_f[0:1, 0:64], rinv[:],
                                     start=True, stop=True)
                    rb_sb = wpool.tile([64, blk], f32, tag="rb_sb")
                    nc.vector.tensor_copy(rb_sb[:], rbp[:])
                    u_sb = wpool.tile([64, blk], f32, tag="u_sb")
                    nc.vector.tensor_tensor(u_sb[:], outp[0:64, :], rb_sb[:],
                                            Alu.mult)
                    e_u = wpool.tile([64, blk], f32, tag="e_u")
                    nc.scalar.activation(e_u[:], u_sb[:], Act.Exp)
                    m_t = wpool.tile([64, blk], f32, tag="m_t")
                    nc.vector.tensor_scalar(m_t[:], e_u[:], -1.0, 0.0,
                                            Alu.add, Alu.min)
                    nc.vector.scalar_tensor_tensor(out_dest, u_sb[:], 0.0,
                                                   m_t[:], Alu.max, Alu.add)

                def gat_scores(hname, src_col_w, src_col_b, dst_all,
                               dst_chunk, wh_chunk, out_dest):
                    # src row of this core's block, then broadcast via PE
                    srp = psm.tile([1, blk], f32, tag="srow")
                    nc.tensor.matmul(srp[:], src_col_w[0], hTb_sb[:, 0, :],
                                     start=True, stop=False)
                    nc.tensor.matmul(srp[:], src_col_w[1], hTb_sb[:, 1, :],
                                     start=False, stop=False)
                    nc.tensor.matmul(srp[:], src_col_b, ones_bf[0:1, 0:blk],
                                     start=False, stop=True)
                    sr_sb = wpool.tile([1, blk], f32, tag="srow_sb")
                    nc.vector.tensor_copy(sr_sb[:], srp[:])
                    sbp = pbc.tile([128, blk], f32, tag="sbc")
                    nc.tensor.matmul(sbp[:], ones_f[0:1, 0:128], sr_sb[:],
                                     start=True, stop=True)
                    score_loop(sbp, dst_all, dst_chunk, wh_chunk, out_dest)

                # ---- layer 1: 4 heads ----
                for h in range(HEADS):
                    cs = HEADS * HID + 2 * h  # src col in wp
                    x2_dest = (x2a_sb if h < 2 else x2b_sb)
                    gat_scores(
                        f"h{h}",
                        (wp_sb[:, 0, cs:cs + 1], wp_sb[:, 1, cs:cs + 1]),
                        wpb_sb[:, cs:cs + 1],
                        sd_sb[:, :, 2 * h + 1],
                        lambda jc, h=h: sd_sb[:, jc, 2 * h + 1:2 * h + 2],
                        lambda jc, h=h: wh_sb[:, jc, h, 0:65],
                        x2_dest[(h % 2) * 64:(h % 2) * 64 + 64, :],
                    )

                # ---- layer 2 projection + AllGather ----
                w2T = pout.tile([OUT_DIM + 2, blk], f32, tag="outp")
                nc.tensor.matmul(w2T[:], w2p_sb[:, 0, :], x2a_sb[:],
                                 start=True, stop=False)
                nc.tensor.matmul(w2T[:], w2p_sb[:, 1, :], x2b_sb[:],
                                 start=False, stop=False)
                nc.tensor.matmul(w2T[:], w2pb_sb[:], ones_bf[0:1, 0:blk],
                                 start=False, stop=True)
                nc.vector.tensor_copy(w2T_sb[:], w2T[:])
                for t in range(nbt):
                    trp = psm.tile([128, OUT_DIM + 2], f32, tag="trp")
                    nc.tensor.transpose(trp[:],
                                        w2T_sb[:, t * 128:(t + 1) * 128],
                                        ident[0:OUT_DIM + 2, 0:OUT_DIM + 2])
                    nc.vector.tensor_copy(agin_sb[:, t, :], trp[:])
                nc.sync.dma_start(ag_in.rearrange("(t p) m -> p t m", p=128),
                                  agin_sb[:])
                nc.gpsimd.collective_compute(
                    "AllGather", Alu.bypass,
                    replica_groups=[list(range(ncores))],
                    ins=[ag_in.opt()], outs=[ag_out[:]])
                agf_sb = cpool.tile([128, njt, OUT_DIM + 2], bf16)
                nc.sync.dma_start(agf_sb[:],
                                  ag_out[:].rearrange("(jc p) m -> p jc m",
                                                      p=128))
                nc.vector.tensor_copy(wh2_sb[:, :, 0:64], agf_sb[:, :, 0:64])
                nc.vector.tensor_copy(dst2_sb[:, :],
                                      agf_sb[:, :, OUT_DIM + 1])

                # ---- layer 2 attention ----
                src2_sb = wpool.tile([1, blk], f32, tag="srow_sb")
                nc.vector.tensor_copy(src2_sb[:],
                                      w2T_sb[OUT_DIM:OUT_DIM + 1, :])

                srp2 = pbc.tile([128, blk], f32, tag="sbc")
                nc.tensor.matmul(srp2[:], ones_f[0:1, 0:128], src2_sb[:],
                                 start=True, stop=True)
                score_loop(srp2, dst2_sb[:, :],
                           lambda jc: dst2_sb[:, jc:jc + 1],
                           lambda jc: wh2_sb[:, jc, 0:65],
                           fin_sb[:])

                # ---- output: transpose back to [blk, 64] ----
                for t in range(nbt):
                    trp = psm.tile([128, OUT_DIM + 2], f32, tag="trp")
                    nc.tensor.transpose(trp[:, 0:OUT_DIM],
                                        fin_sb[:, t * 128:(t + 1) * 128],
                                        ident[0:64, 0:64])
                    nc.vector.tensor_copy(ob_sb[:, t, :], trp[:, 0:OUT_DIM])
                nc.sync.dma_start(out_d.rearrange("(t p) d -> p t d", p=128),
                                  ob_sb[:])

    nc.compile()
    return nc


def _host_prep(h, adj, Wh, bWh, a_src, a_dst, a_b, Wo, bWo, ao_src, ao_dst,
               ao_b, n=N, ncores=NCORES):
    """Build per-core input maps (numpy only)."""
    blk = n // ncores
    h = np.asarray(h, np.float32)
    adj = np.asarray(adj)

    hT = np.ascontiguousarray(h.T).astype(BF16)                    # [256, n]

    wp = np.zeros((IN_DIM + 1, HEADS * HID + 2 * HEADS), np.float32)
    for hd in range(HEADS):
        W = np.asarray(Wh[hd], np.float32)
        wp[0:IN_DIM, hd * HID:(hd + 1) * HID] = W
        wp[IN_DIM, hd * HID:(hd + 1) * HID] = np.asarray(bWh[hd], np.float32)
        asr = np.asarray(a_src[hd], np.float32)
        ads = np.asarray(a_dst[hd], np.float32)
        cs = HEADS * HID + 2 * hd
        wp[0:IN_DIM, cs] = W @ asr
        wp[IN_DIM, cs] = float(np.asarray(bWh[hd], np.float32) @ asr) + float(
            np.asarray(a_b[hd], np.float32))
        wp[0:IN_DIM, cs + 1] = W @ ads
        wp[IN_DIM, cs + 1] = float(np.asarray(bWh[hd], np.float32) @ ads)
    wp = wp.astype(BF16)

    Wo = np.asarray(Wo, np.float32)
    w2p = np.zeros((HEADS * HID + 1, OUT_DIM + 2), np.float32)
    w2p[0:HEADS * HID, 0:OUT_DIM] = Wo
    w2p[HEADS * HID, 0:OUT_DIM] = np.asarray(bWo, np.float32)
    aos = np.asarray(ao_src, np.float32)
    aod = np.asarray(ao_dst, np.float32)
    w2p[0:HEADS * HID, OUT_DIM] = Wo @ aos
    w2p[HEADS * HID, OUT_DIM] = float(np.asarray(bWo, np.float32) @ aos) + \
        float(np.asarray(ao_b, np.float32))
    w2p[0:HEADS * HID, OUT_DIM + 1] = Wo @ aod
    w2p[HEADS * HID, OUT_DIM + 1] = float(np.asarray(bWo, np.float32) @ aod)
    w2p = w2p.astype(BF16)

    adjf = (adj > 0).astype(np.float32)

    in_maps = []
    for c in range(ncores):
        sl = slice(c * blk, (c + 1) * blk)
        in_maps.append({
            "hT": hT,
            "hTb": np.ascontiguousarray(hT[:, sl]),
            "wp": wp,
            "w2p": w2p,
            "adjm": np.ascontiguousarray(adjf[sl, :].T).astype(BF16),
        })
    return in_maps


_NC_CACHE = {}


def _get_program(n=N, ncores=NCORES):
    key = (n, ncores, ACT_FRAC, GP_FRAC, USE_PRELU)
    if key not in _NC_CACHE:
        _NC_CACHE[key] = _build_program(n, ncores, ACT_FRAC, GP_FRAC,
                                        USE_PRELU)
    return _NC_CACHE[key]


def kernel(**inputs):
    from concourse.bass_utils import run_bass_kernel_spmd

    nc = _get_program()
    in_maps = _host_prep(**inputs)
    res = run_bass_kernel_spmd(nc, in_maps, core_ids=list(range(NCORES)))
    out = np.concatenate([r["out_block"] for r in res.results], axis=0)
    return out.astype(np.float32)


# ---------------------------------------------------------------------------
# Self-test on the CPU simulator with a small N (no hardware needed).
def _np_reference(h, adj, Wh, bWh, a_src, a_dst, a_b, Wo, bWo, ao_src,
                  ao_dst, ao_b):
    NEG = -9e15

    def layer(x, mask, W, b, asr, ads, ab):
        wh = x @ W + b
        s = (wh @ asr)[:, None] + (wh @ ads)[None, :] + ab
        s = np.where(s >= 0, s, 0.2 * s)
        s = np.where(mask, s, NEG)
        s = s - s.max(axis=1, keepdims=True)
        e = np.exp(s)
        att = e / e.sum(axis=1, keepdims=True)
        return att @ wh

    def elu(x):
        return np.where(x > 0, x, np.exp(np.minimum(x, 0)) - 1)

    mask = adj > 0
    heads = [layer(h, mask, Wh[i], bWh[i], a_src[i], a_dst[i], a_b[i])
             for i in range(HEADS)]
    x2 = np.concatenate([elu(hh) for hh in heads], axis=1)
    return elu(layer(x2, mask, Wo, bWo, ao_src, ao_dst, ao_b))


def _selftest(n=1024):
    from concourse.bass_interp import MultiCoreSim

    rng = np.random.default_rng(0)
    h = rng.standard_normal((n, IN_DIM)).astype(np.float32)
    adj = rng.integers(0, 2, (n, n)).astype(np.int32)
    Wh = (rng.standard_normal((HEADS, IN_DIM, HID)) * 0.08).astype(np.float32)
    bWh = (rng.standard_normal((HEADS, HID)) * 0.05).astype(np.float32)
    a_src = (rng.standard_normal((HEADS, HID)) * 0.1).astype(np.float32)
    a_dst = (rng.standard_normal((HEADS, HID)) * 0.1).astype(np.float32)
    a_b = (rng.standard_normal((HEADS,)) * 0.05).astype(np.float32)
    Wo = (rng.standard_normal((HEADS * HID, OUT_DIM)) * 0.06).astype(np.float32)
    bWo = (rng.standard_normal((OUT_DIM,)) * 0.05).astype(np.float32)
    ao_src = (rng.standard_normal((OUT_DIM,)) * 0.1).astype(np.float32)
    ao_dst = (rng.standard_normal((OUT_DIM,)) * 0.1).astype(np.float32)
    ao_b = np.float32(0.03)

    inputs = dict(h=h, adj=adj, Wh=Wh, bWh=bWh, a_src=a_src, a_dst=a_dst,
                  a_b=a_b, Wo=Wo, bWo=bWo, ao_src=ao_src, ao_dst=ao_dst,
                  ao_b=ao_b)
    expected = _np_reference(**inputs)

    nc = _build_program(n=n, ncores=NCORES, use_prelu=False)
    in_maps = _host_prep(n=n, ncores=NCORES, **inputs)
    sim = MultiCoreSim(nc, num_cores=NCORES)
    for c in range(NCORES):
        for k, v in in_maps[c].items():
            sim.cores[c].tensor(k)[:] = v
    sim.simulate(check_with_hw=False)
    out = np.concatenate(
        [np.asarray(sim.cores[c].tensor("out_block")) for c in range(NCORES)],
        axis=0)

    err = np.abs(out - expected)
    scale = np.abs(expected).max()
    print("max abs err:", err.max(), "scale:", scale,
          "rel:", err.max() / scale)
    return err.max() / scale


if __name__ == "__main__":
    _selftest(int(sys.argv[1]) if len(sys.argv) > 1 else 1024)
